# revision 1
# baseline (speedup 1.0000x reference)
"""BertAttention Trainium2 kernel (8 NeuronCores, SPMD).

Sharding: core c handles batch b = c//2 and head-half hh = c%2 (8 of 16 heads).
Each core computes q/k/v projections for its 512 head-dims over its batch's
full sequence, per-head attention (no mask, scale 1/sqrt(1024)), and a partial
o-projection over its 512 context dims.  Host sums the two partials per batch.

Device layout (per core):
  xt   [8,128,2048]  f32r  hidden[b].T, d-major chunks
  wqt  [4,8,128,128] f32r  w_q rows for our heads, transposed, (pair, k) chunks
  wkt  [4,8,128,128] f32r  same for w_k
  wvt  [2,8,128,256] f32r  w_v rows transposed, (col-half, k) chunks
  wot  [4,128,1024]  f32r  w_o cols for our heads, transposed, pair chunks
  outt [8,128,2048]  f32   out_partial.T (o-major chunks)

Attention per head-pair p (heads 2p, 2p+1 local):
  QT/KT [128, 2048] = heads' q/k transposed (head on partitions 0:64 / 64:128)
  S^T tile [128k, 2, 512q]: two row-packed matmuls (K=64 at base 0 and 64)
  exp: one activation over [128, 1024] with fused 1/32 scale -> f32r
  AV: per head, lhsT = V_aug[kt][:, head, 0:65] (64 v-cols + ones col),
      rhs = P^T chunk, accumulated over 16 k-tiles -> psum [65, 512]
      row 64 = softmax denominator.
  norm: den row -> ones-matmul broadcast [64,512] -> DVE recip -> DVE mult
"""

import sys

sys.path.insert(0, "/opt/trn_rl_repo")

import numpy as np

B, S, D, H = 4, 2048, 1024, 16
HEAD = 64
NCORES = 8
P = 128
NQ = 512            # q free-tile width
KT_TILES = S // P   # 16 k tiles
QT_TILES = S // NQ  # 4 q tiles
DC = 8              # contraction chunks for projections (1024/128)
PAIRS = 4           # head pairs per core


def _round_fp32r(x: np.ndarray) -> np.ndarray:
    """RNE-round fp32 to the 11-bit-mantissa fp32r grid (low 12 bits zero)."""
    b = np.ascontiguousarray(x, np.float32).view(np.uint32).astype(np.uint64)
    lsb = (b >> 12) & 1
    b = b + 0x7FF + lsb
    b = (b & 0xFFFFF000) & 0xFFFFFFFF
    return b.astype(np.uint32).view(np.float32)


_NC_CACHE = None


def _build_nc():
    import concourse.bass as bass  # noqa: F401
    import concourse.tile as tile
    from concourse import bacc, mybir

    f32 = mybir.dt.float32
    f32r = mybir.dt.float32r
    f16 = mybir.dt.float16
    AF = mybir.ActivationFunctionType

    nc = bacc.Bacc(None)
    xt_d = nc.declare_dram_parameter("xt", [DC, P, S], f16, isOutput=False)
    wqt_d = nc.declare_dram_parameter("wqt", [PAIRS, DC, P, P], f16, isOutput=False)
    wkt_d = nc.declare_dram_parameter("wkt", [PAIRS, DC, P, P], f16, isOutput=False)
    wvt_d = nc.declare_dram_parameter("wvt", [DC, P, 512], f16, isOutput=False)
    woth_d = nc.declare_dram_parameter("woth", [PAIRS, P, D], f16, isOutput=False)
    out_d = nc.declare_dram_parameter("outt", [D // P, P, S], f32, isOutput=True)

    from contextlib import ExitStack

    with tile.TileContext(nc) as tc, ExitStack() as es:
        def pool(name, bufs, space="SBUF"):
            return es.enter_context(
                tc.tile_pool(name=name, bufs=bufs, space=space))

        xt_pool = pool("xt", 1)
        wq_pool = pool("wq", 8)
        wk_pool = pool("wk", 8)
        wv_pool = pool("wv", 8)
        qt_pool = pool("qt", 2)
        kt_pool = pool("kt", 2)
        v_pool = pool("v", 1)
        pt_pool = pool("pt", 4)
        ctx_pool = pool("ctx", 1)
        wot_pool = pool("wot", 1)
        ost_pool = pool("ost", 1)
        dn_pool = pool("dn", 1)
        rc_pool = pool("rc", 1)
        on_pool = pool("on", 1)
        pp_pool = pool("pp", 2, "PSUM")
        st_pool = pool("st", 2, "PSUM")
        av_pool = pool("av", 2, "PSUM")

        if True:
            # ones row (f32r) for the denominator broadcast matmul
            ones_f = on_pool.tile([P, P], f32, tag="onf", name="onesf")
            nc.vector.memset(ones_f[:], 1.0)
            ones_r = on_pool.tile([P, P], f32r, tag="onr", name="onesr")
            nc.vector.tensor_copy(ones_r[:], ones_f[:])

            # PE warmup during the initial DMA: keeps HAM at 8/8 so the
            # first projection matmuls run at 2.4 GHz
            wup = on_pool.tile([P, NQ], f16, tag="wup", name="wup")
            nc.vector.memset(wup[:], 0.125)
            wups = pp_pool.tile([P, NQ], f32, tag="pp", name="wups")
            for _ in range(30):
                nc.tensor.matmul(wups[:], wup[:, 0:P], wup[:],
                                 start=True, stop=True)

            # load x^T chunks
            xt = []
            for k in range(DC):
                t = xt_pool.tile([P, S], f16, tag=f"xt{k}", name=f"xt{k}")
                eng = nc.sync if k % 2 == 0 else nc.scalar
                eng.dma_start(t[:], xt_d[k])
                xt.append(t)

            # V_aug: separate tiles per head-half (heads 4h..4h+3); ones col
            # per head at offset 65h+64.  One N=512 projection pass fills both.
            v_half = {0: [None] * KT_TILES, 1: [None] * KT_TILES}
            wv_t = []

            def load_wv():
                for k in range(DC):
                    t = wv_pool.tile([P, NQ], f16, tag="wv", name="wv")
                    nc.scalar.dma_start(t[:], wvt_d[k])
                    wv_t.append(t)

            def proj_v(mts):
                for mt in mts:
                    ps = pp_pool.tile([P, NQ], f32, tag="pp", name="pp")
                    for k in range(DC):
                        nc.tensor.matmul(
                            ps[:], xt[k][:, mt * P:(mt + 1) * P], wv_t[k][:],
                            start=(k == 0), stop=(k == DC - 1),
                        )
                    for half in range(2):
                        t = v_pool.tile([P, 4, 65], f16, tag=f"v{half}_{mt}",
                                        name=f"v{half}_{mt}")
                        nc.vector.memset(t[:], 1.0)
                        v_half[half][mt] = t
                        src = ps[:, half * 256:half * 256 + 256].rearrange(
                            "p (h d) -> p h d", h=4)
                        nc.vector.tensor_copy(t[:, :, 0:64], src)

            def load_w(w_pool, w_dram, p):
                w_t = []
                for k in range(DC):
                    t = w_pool.tile([P, P], f16, tag="w", name="w")
                    nc.sync.dma_start(t[:], w_dram[p, k])
                    w_t.append(t)
                return w_t

            def proj_nt(out, w_t, nt):
                ps = pp_pool.tile([P, NQ], f32, tag="pp", name="pp")
                for k in range(DC):
                    nc.tensor.matmul(
                        ps[:], w_t[k][:], xt[k][:, nt * NQ:(nt + 1) * NQ],
                        start=(k == 0), stop=(k == DC - 1),
                    )
                nc.vector.tensor_copy(out[:, nt * NQ:(nt + 1) * NQ], ps[:])

            def attention(p, QT, KT, ctx_p, post_qt=None, pre_kt=None):
                for qt in range(QT_TILES):
                    avs = [av_pool.tile([65, NQ], f32, tag="av", name=f"av{j}") for j in range(2)]

                    def av_mms(pt, kt):
                        vh = v_half[p // 2]
                        for j in range(2):
                            nc.tensor.matmul(
                                avs[j][:], vh[kt][:, (2 * p + j) % 4, :],
                                pt[:, j, :],
                                start=(kt == 0), stop=(kt == KT_TILES - 1),
                                skip_group_check=True,
                            )

                    prev = None
                    for kt in range(KT_TILES):
                        if pre_kt is not None and qt == 0:
                            pre_kt(kt)
                        st = st_pool.tile([P, 2, NQ], f32, tag="st", name="st")
                        for j in range(2):
                            h = j * 64
                            nc.tensor.matmul(
                                st[:, j, :],
                                KT[h:h + 64, kt * P:(kt + 1) * P],
                                QT[h:h + 64, qt * NQ:(qt + 1) * NQ],
                                start=True, stop=True,
                            )
                        pt = pt_pool.tile([P, 2, NQ], f16, tag="pt", name="pt")
                        nc.scalar.activation(pt[:], st[:], AF.Exp, scale=0.03125)
                        if prev is not None:
                            av_mms(prev, kt - 1)
                        prev = pt
                    av_mms(prev, KT_TILES - 1)
                    for j in range(2):
                        ctx_slice = ctx_p[j * 64:(j + 1) * 64,
                                          qt * NQ:(qt + 1) * NQ]
                        den = dn_pool.tile([65, NQ], f32r, tag="dn", name="dn")
                        nc.vector.tensor_copy(den[64:65, :], avs[j][64:65, :])
                        # unnormalized ctx out of PSUM (releases the av slot
                        # without waiting on the reciprocal chain)
                        nc.vector.tensor_copy(ctx_slice, avs[j][0:64, :])
                        bc = av_pool.tile([P, NQ], f32, tag="av", name="bc")
                        nc.tensor.matmul(
                            bc[:], ones_r[64:65, :], den[64:65, :],
                            start=True, stop=True,
                        )
                        rec = rc_pool.tile([P, NQ], f32, tag="rc", name="rec")
                        nc.vector.reciprocal_approx_fast(rec[:], bc[:])
                        nc.vector.tensor_mul(
                            out=ctx_slice,
                            in0=ctx_slice,
                            in1=rec[j * 64:(j + 1) * 64, :],
                        )
                    if post_qt is not None:
                        post_qt(qt)

            # pipeline: proj(0), proj(1), att(0), proj(2)+oproj(0), ...
            QTs, KTs, ctxs = {}, {}, {}
            out_sb = []
            wot_t = {}
            for _ot in range(D // P):
                _t = ost_pool.tile([P, S], f32, tag=f"ou{_ot}", name=f"ou{_ot}")
                out_sb.append(_t)

            def proj_pair(p):
                wk_t = load_w(wk_pool, wkt_d, p)
                wq_t = load_w(wq_pool, wqt_d, p)
                KT = kt_pool.tile([P, S], f16, tag="t", name="kt_t")
                QT = qt_pool.tile([P, S], f16, tag="t", name="qt_t")
                for nt in range(QT_TILES):
                    proj_nt(KT, wk_t, nt)
                    proj_nt(QT, wq_t, nt)
                KTs[p], QTs[p] = KT, QT
                ctx_t = ctx_pool.tile([P, S], f16, tag=f"ctx{p}", name=f"ctx{p}")
                ctxs[p] = ctx_t

            def load_wot(p):
                th = wot_pool.tile([P, D], f16, tag=f"woth{p}", name=f"woth{p}")
                nc.sync.dma_start(th[:], woth_d[p])
                wot_t[p] = (th,)

            def oproj_chunk(p, qt, dma_out=False):
                for ot in range(D // P):
                    ps = pp_pool.tile([P, NQ], f32, tag="pp", name="pp")
                    ws = wot_t[p]
                    for i_mm, w in enumerate(ws):
                        nc.tensor.matmul(
                            ps[:], w[:, ot * P:(ot + 1) * P],
                            ctxs[p][:, qt * NQ:(qt + 1) * NQ],
                            start=(i_mm == 0), stop=(i_mm == len(ws) - 1),
                        )
                    dst = out_sb[ot][:, qt * NQ:(qt + 1) * NQ]
                    if p == 0:
                        nc.vector.tensor_copy(dst, ps[:])
                    else:
                        nc.vector.tensor_add(dst, dst, ps[:])
                    if dma_out:
                        nc.sync.dma_start(out_d[ot][:, qt * NQ:(qt + 1) * NQ],
                                          dst)

            def oproj_pair(p):
                for qt in range(QT_TILES):
                    oproj_chunk(p, qt)

            load_wv()
            proj_pair(0)
            attention(0, QTs[0], KTs[0], ctxs[0],
                      pre_kt=lambda kt: proj_v([kt]))
            proj_pair(1)
            attention(1, QTs[1], KTs[1], ctxs[1])
            proj_pair(2)
            load_wot(0)
            oproj_pair(0)
            attention(2, QTs[2], KTs[2], ctxs[2])
            proj_pair(3)
            load_wot(1)
            oproj_pair(1)
            load_wot(2)
            load_wot(3)
            attention(3, QTs[3], KTs[3], ctxs[3],
                      post_qt=lambda qt: (oproj_chunk(2, qt),
                                          oproj_chunk(3, qt, dma_out=True)))



    nc.finalize()
    return nc


def _get_nc():
    global _NC_CACHE
    if _NC_CACHE is None:
        _NC_CACHE = _build_nc()
    return _NC_CACHE


def _make_in_maps(hidden_state, w_q, w_k, w_v, w_o):
    hidden_state = np.asarray(hidden_state, np.float32)
    w_q = np.asarray(w_q, np.float32)
    w_k = np.asarray(w_k, np.float32)
    w_v = np.asarray(w_v, np.float32)
    w_o = np.asarray(w_o, np.float32)

    in_maps = []
    for core in range(NCORES):
        b, hh = core // 2, core % 2
        rows = slice(hh * 512, (hh + 1) * 512)
        xt = hidden_state[b].T.astype(np.float16).reshape(DC, P, S)
        # w[rows].T: [1024 d, 512 c] -> (pair, k) chunks [4, 8, 128, 128]
        wqt = (w_q[rows].T.reshape(DC, P, PAIRS, P).transpose(2, 0, 1, 3)
               .astype(np.float16))
        wkt = (w_k[rows].T.reshape(DC, P, PAIRS, P).transpose(2, 0, 1, 3)
               .astype(np.float16))
        wvt = w_v[rows].T.reshape(DC, P, 512).astype(np.float16)
        wot = np.ascontiguousarray(w_o[:, rows].T.reshape(PAIRS, P, D),
                                   np.float32)
        woth = wot.astype(np.float16)
        in_maps.append({"xt": np.ascontiguousarray(xt),
                        "wqt": np.ascontiguousarray(wqt),
                        "wkt": np.ascontiguousarray(wkt),
                        "wvt": np.ascontiguousarray(wvt),
                        "woth": woth})
    return in_maps


def _assemble(results):
    out = np.empty((B, S, D), np.float32)
    for b in range(B):
        t = (results[2 * b]["outt"].reshape(D, S).astype(np.float32)
             + results[2 * b + 1]["outt"].reshape(D, S).astype(np.float32))
        out[b] = t.T
    return out


def run_spmd(hidden_state, w_q, w_k, w_v, w_o, **spmd_kwargs):
    """Run the kernel; returns (output, BassKernelResults)."""
    from concourse.bass_utils import run_bass_kernel_spmd

    nc = _get_nc()
    in_maps = _make_in_maps(hidden_state, w_q, w_k, w_v, w_o)
    res = run_bass_kernel_spmd(nc, in_maps, core_ids=list(range(NCORES)),
                               **spmd_kwargs)
    return _assemble(res.results), res


def kernel(hidden_state, attention_mask=None, w_q=None, w_k=None, w_v=None,
           w_o=None):
    out, _ = run_spmd(hidden_state, w_q, w_k, w_v, w_o)
    return out



# revision 5
# speedup vs baseline: 1.0882x; 1.0882x over previous
"""BertAttention Trainium2 kernel (8 NeuronCores, SPMD).

Sharding: core c handles batch b = c//2 and head-half hh = c%2 (8 of 16 heads).
Each core computes q/k/v projections for its 512 head-dims over its batch's
full sequence, per-head attention (no mask, scale 1/sqrt(1024)), and a partial
o-projection over its 512 context dims.  Host sums the two partials per batch.

v2 design (ACT-exp is the hard floor at ~285us/core; PE work reduced below it):
  QK  : unchanged (K=64 row-packed pairs, fp16, half-rate -- irreducible).
  exp : ACT over st [128,2,512] psum -> pt [128,2,512] f16, 256 instrs.
  AV  : SWAPPED dataflow -- stationary = P^T chunk [128k,128q] (from pt),
        moving = V_aug [128k,65] (64 v-dims + ones col). out = ctx [128q,65]
        psum, accumulated over 16 k-tiles per (head j, q-chunk qc).  Full
        128-wide array => ~2x fewer PE cycles than the ctx^T form.  The
        softmax denominator lands in psum COLUMN 64 -> per-partition DVE
        reciprocal + tensor_scalar_mul (no broadcast matmuls, no [64,2048]
        DVE multiplies).
  ctxT: PE transpose (identity matmul) of normalized ctx [128q,64] ->
        [64,128] written at column-position 64j, gpsimd copies psum->sbuf.
  oproj: as baseline (per-pair psum chunks, DVE add into out_sb), fp16 out.

Emission order keeps ACT continuously fed: per (pair p, qt) block, the 16
kt-slots emit QK+exp, and between them: AV chains of the PREVIOUS block
(1 chain per 2 slots), q/k projection bursts for pair p+1, v-projection
half-passes, and o-proj chunks for pair p-1.

PSUM (16KB/partition): st 2x[128,2,512]f32 (8KB) + acc 2x[128,65]f32 padded
to 2KB (4KB) + pp 2x2KB (proj/oproj/transpose, 4KB).
"""

import sys

sys.path.insert(0, "/opt/trn_rl_repo")

import numpy as np

B, S, D, H = 4, 2048, 1024, 16
HEAD = 64
NCORES = 8
P = 128
NQ = 512            # q free-tile width
KT_TILES = S // P   # 16 k tiles
QT_TILES = S // NQ  # 4 q tiles
DC = 8              # contraction chunks for projections (1024/128)
PAIRS = 4           # head pairs per core


_NC_CACHE = None


def _build_nc():
    import concourse.bass as bass  # noqa: F401
    import concourse.tile as tile
    from concourse import bacc, mybir

    f32 = mybir.dt.float32
    f16 = mybir.dt.float16
    AF = mybir.ActivationFunctionType

    nc = bacc.Bacc(None)
    xt_d = nc.declare_dram_parameter("xt", [DC, P, S], f16, isOutput=False)
    wqt_d = nc.declare_dram_parameter("wqt", [PAIRS, DC, P, P], f16, isOutput=False)
    wkt_d = nc.declare_dram_parameter("wkt", [PAIRS, DC, P, P], f16, isOutput=False)
    wvt_d = nc.declare_dram_parameter("wvt", [DC, P, 512], f16, isOutput=False)
    woth_d = nc.declare_dram_parameter("woth", [PAIRS, P, D], f16, isOutput=False)
    ident_d = nc.declare_dram_parameter("ident", [P, P], f16, isOutput=False)
    out_d = nc.declare_dram_parameter("outt", [D // P, P, S], f16, isOutput=True)

    from contextlib import ExitStack

    with tile.TileContext(nc) as tc, ExitStack() as es:
        def pool(name, bufs, space="SBUF"):
            return es.enter_context(
                tc.tile_pool(name=name, bufs=bufs, space=space))

        xt_pool = pool("xt", 1)
        # 16 slots = 2 pairs' weights resident: pair p+1's weight DMAs must
        # not wait on pair p's LAST (late-dribbled) proj burst, since pair
        # p+1's own bursts sit earlier in the in-order PE stream.
        wq_pool = pool("wq", 16)
        wk_pool = pool("wk", 16)
        wv_pool = pool("wv", 8)
        qt_pool = pool("qt", 2)
        kt_pool = pool("kt", 2)
        v_pool = pool("v", 1)
        pt_pool = pool("pt", 32)
        cn_pool = pool("cn", 4)
        rc_pool = pool("rc", 4)
        ctx_pool = pool("ctx", 1)
        wot_pool = pool("wot", 1)
        ost_pool = pool("ost", 1)
        on_pool = pool("on", 1)
        pp_pool = pool("pp", 2, "PSUM")
        st_pool = pool("st", 2, "PSUM")
        acc_pool = pool("acc", 2, "PSUM")

        # identity for PE transposes
        ident = on_pool.tile([P, P], f16, tag="id", name="ident")
        nc.sync.dma_start(ident[:], ident_d[:, :])

        # PE warmup during the initial DMA: keeps HAM at 8/8 so the
        # first projection matmuls run at 2.4 GHz
        wup = on_pool.tile([P, NQ], f16, tag="wup", name="wup")
        nc.vector.memset(wup[:], 0.125)
        wups = pp_pool.tile([P, NQ], f32, tag="pp", name="wups")
        for _ in range(30):
            nc.tensor.matmul(wups[:], wup[:, 0:P], wup[:],
                             start=True, stop=True)

        # load x^T chunks
        xt = []
        for k in range(DC):
            t = xt_pool.tile([P, S], f16, tag=f"xt{k}", name=f"xt{k}")
            eng = nc.sync if k % 2 == 0 else nc.gpsimd
            eng.dma_start(t[:], xt_d[k])
            xt.append(t)

        wv_t = []
        for k in range(DC):
            t = wv_pool.tile([P, NQ], f16, tag="wv", name="wv")
            nc.gpsimd.dma_start(t[:], wvt_d[k])
            wv_t.append(t)

        # V_aug tiles per k-tile: [128 keys, 4 heads, 65] (64 v-dims + ones)
        v_half = {0: [None] * KT_TILES, 1: [None] * KT_TILES}

        def proj_v(mt, half):
            """one N=256 projection pass filling v_half[half][mt]"""
            ps = pp_pool.tile([P, 256], f32, tag="pp", name="ppv")
            for k in range(DC):
                nc.tensor.matmul(
                    ps[:], xt[k][:, mt * P:(mt + 1) * P],
                    wv_t[k][:, half * 256:(half + 1) * 256],
                    start=(k == 0), stop=(k == DC - 1),
                )
            t = v_pool.tile([P, 4, 65], f16, tag=f"v{half}_{mt}",
                            name=f"v{half}_{mt}")
            nc.vector.memset(t[:], 1.0)
            v_half[half][mt] = t
            src = ps[:, :].rearrange("p (h d) -> p h d", h=4)
            nc.vector.tensor_copy(t[:, :, 0:64], src)

        def load_w(w_pool, w_dram, p):
            w_t = []
            for k in range(DC):
                t = w_pool.tile([P, P], f16, tag="w", name="w")
                nc.sync.dma_start(t[:], w_dram[p, k])
                w_t.append(t)
            return w_t

        def proj_nt(out, w_t, nt):
            ps = pp_pool.tile([P, NQ], f32, tag="pp", name="pp")
            for k in range(DC):
                nc.tensor.matmul(
                    ps[:], w_t[k][:], xt[k][:, nt * NQ:(nt + 1) * NQ],
                    start=(k == 0), stop=(k == DC - 1),
                )
            nc.vector.tensor_copy(out[:, nt * NQ:(nt + 1) * NQ], ps[:])

        # ---- per-pair state ----
        QTs, KTs, ctxTs = {}, {}, {}
        wot_t = {}
        wqk_t = {}
        out_sb = []
        for _ot in range(D // P):
            _t = ost_pool.tile([P, S], f16, tag=f"ou{_ot}", name=f"ou{_ot}")
            out_sb.append(_t)

        def load_wot(p):
            th = wot_pool.tile([P, D], f16, tag=f"woth{p}", name=f"woth{p}")
            nc.gpsimd.dma_start(th[:], woth_d[p])
            wot_t[p] = th

        def alloc_pair(p):
            KTs[p] = kt_pool.tile([P, S], f16, tag="t", name=f"kt{p}")
            QTs[p] = qt_pool.tile([P, S], f16, tag="t", name=f"qt{p}")
            ctxTs[p] = ctx_pool.tile([P, S], f16, tag=f"ctx{p}",
                                     name=f"ctx{p}")

        def oproj_chunk(p, qt):
            for ot in range(D // P):
                ps = pp_pool.tile([P, NQ], f32, tag="pp", name="pp")
                nc.tensor.matmul(
                    ps[:], wot_t[p][:, ot * P:(ot + 1) * P],
                    ctxTs[p][:, qt * NQ:(qt + 1) * NQ],
                    start=True, stop=True,
                )
                dst = out_sb[ot][:, qt * NQ:(qt + 1) * NQ]
                if p == 0:
                    nc.vector.tensor_copy(dst, ps[:])
                else:
                    nc.vector.tensor_add(dst, dst, ps[:])
                if p == PAIRS - 1:
                    nc.sync.dma_start(out_d[ot][:, qt * NQ:(qt + 1) * NQ],
                                      dst)

        # pt tiles of the two most recent blocks
        pt_map = {}

        tp_box = [None]

        def av_chain(bp, bqt, c):
            """AV chain c (j = c%2, qc = c//2) of block (bp, bqt):
            ctx[q,d] accumulation + normalize + transpose into ctxT."""
            j, qc = c % 2, c // 2
            half = bp // 2
            jj = (2 * bp + j) % 4
            acc = acc_pool.tile([P, 65], f32, tag="acc", name="acc",
                                padded_shape=[P, 512])
            for i in range(KT_TILES):
                nc.tensor.matmul(
                    acc[:],
                    pt_map[(bp, bqt, i)][:, j, qc * P:(qc + 1) * P],
                    v_half[half][i][:, jj, :],
                    start=(i == 0), stop=(i == KT_TILES - 1),
                    skip_group_check=True,
                )
            rec = rc_pool.tile([P, 1], f32, tag="rc", name="rec")
            nc.vector.reciprocal_approx_fast(rec[:], acc[:, 64:65])
            ctxn = cn_pool.tile([P, 64], f16, tag="cn", name="ctxn")
            nc.vector.tensor_scalar_mul(ctxn[:], acc[:, 0:64], rec[:, 0:1])
            if j == 0:
                tp_box[0] = pp_pool.tile([P, P], f16, tag="pp", name="tp")
            tp = tp_box[0]
            nc.tensor.matmul(tp[64 * j:64 * (j + 1), :], ctxn[:], ident[:],
                             is_transpose=True)
            if j == 1:
                # gpsimd cannot read PSUM; DVE does the psum->sbuf hop
                nc.vector.tensor_copy(
                    ctxTs[bp][:, bqt * NQ + qc * P: bqt * NQ + (qc + 1) * P],
                    tp[:],
                )

        def qk_slot(p, qt, kt):
            st = st_pool.tile([P, 2, NQ], f32, tag="st", name="st")
            for j in range(2):
                h = j * 64
                nc.tensor.matmul(
                    st[:, j, :],
                    KTs[p][h:h + 64, kt * P:(kt + 1) * P],
                    QTs[p][h:h + 64, qt * NQ:(qt + 1) * NQ],
                    start=True, stop=True,
                )
            pt = pt_pool.tile([P, 2, NQ], f16, tag="pt", name="pt")
            nc.scalar.activation(pt[:], st[:], AF.Exp, scale=0.03125)
            pt_map[(p, qt, kt)] = pt

        # ---------------- schedule ----------------
        # extra-work placement: dict (p, qt, kt) -> list of callables
        extra = {}

        def put(p, qt, kt, fn):
            extra.setdefault((p, qt, kt), []).append(fn)

        # pair 0 projections: K nt0 + Q nt0 upfront; rest dribbled in (0,0)
        alloc_pair(0)
        wqk_t[0] = (load_w(wk_pool, wkt_d, 0), load_w(wq_pool, wqt_d, 0))
        proj_nt(KTs[0], wqk_t[0][0], 0)
        proj_nt(QTs[0], wqk_t[0][1], 0)
        # remaining K-chunks of pair 0 needed at kt = 4*nt of every block
        for ntc in (1, 2, 3):
            put(0, 0, 4 * (ntc - 1) + 1,
                lambda ntc=ntc: proj_nt(KTs[0], wqk_t[0][0], ntc))
        # Q-chunks of pair 0: Qnt_c needed from block (0, c)
        for ntc in (1, 2, 3):
            put(0, ntc - 1, 6, lambda ntc=ntc: proj_nt(QTs[0], wqk_t[0][1], ntc))
        # v-projection half 0 (pairs 0,1): all 16 passes in block (0,0) --
        # the first AV chains (of block (0,0), run in block (0,1)) need
        # every v tile, so none may be emitted later.
        for mt in range(KT_TILES):
            put(0, 0, mt, lambda mt=mt: proj_v(mt, 0))
        # v-projection half 1 (pairs 2,3): spread over pair-1 blocks
        for mt in range(KT_TILES):
            put(1, mt // 4, (mt % 4) * 4 + 2, lambda mt=mt: proj_v(mt, 1))

        # pair p+1 q/k projections: 8 bursts spread over pair-p blocks
        for p in range(PAIRS - 1):
            pn = p + 1

            def loadw(pn=pn):
                alloc_pair(pn)
                wqk_t[pn] = (load_w(wk_pool, wkt_d, pn),
                             load_w(wq_pool, wqt_d, pn))
            put(p, 0, 0, loadw)
            for i in range(4):  # K chunks first
                put(p, 1 + i % 3, 3 + 2 * (i // 3),
                    lambda pn=pn, i=i: proj_nt(KTs[pn], wqk_t[pn][0], i))
            for i in range(4):
                put(p, 1 + i % 3, 9 + 2 * (i // 3),
                    lambda pn=pn, i=i: proj_nt(QTs[pn], wqk_t[pn][1], i))

        # wot loads + o-proj chunks for pair p-1 during pair p's blocks
        put(0, 2, 1, lambda: load_wot(0))
        put(1, 0, 1, lambda: load_wot(1))
        put(2, 0, 1, lambda: load_wot(2))
        put(2, 2, 1, lambda: load_wot(3))
        for p in range(1, PAIRS):
            for qt in range(QT_TILES):
                put(p, qt, 13, lambda p=p, qt=qt: oproj_chunk(p - 1, qt))
        # pair 3's own o-proj: chunk qt right after its chains complete
        for qt in range(1, QT_TILES):
            put(3, qt, 15, lambda qt=qt: oproj_chunk(3, qt - 1))

        blocks = [(p, qt) for p in range(PAIRS) for qt in range(QT_TILES)]
        for bi, (p, qt) in enumerate(blocks):
            prev = blocks[bi - 1] if bi > 0 else None
            for kt in range(KT_TILES):
                qk_slot(p, qt, kt)
                if prev is not None and kt % 2 == 1:
                    av_chain(prev[0], prev[1], kt // 2)
                for fn in extra.get((p, qt, kt), ()):
                    fn()
            if prev is not None:
                # release prev block's pt tiles from the map
                for kk in range(KT_TILES):
                    del pt_map[(prev[0], prev[1], kk)]
        # tail: chains of the last block + final o-proj chunk
        for c in range(8):
            av_chain(3, 3, c)
        oproj_chunk(3, 3)

    nc.finalize()
    return nc


def _get_nc():
    global _NC_CACHE
    if _NC_CACHE is None:
        _NC_CACHE = _build_nc()
    return _NC_CACHE


def _make_in_maps(hidden_state, w_q, w_k, w_v, w_o):
    hidden_state = np.asarray(hidden_state, np.float32)
    w_q = np.asarray(w_q, np.float32)
    w_k = np.asarray(w_k, np.float32)
    w_v = np.asarray(w_v, np.float32)
    w_o = np.asarray(w_o, np.float32)

    ident = np.eye(P, dtype=np.float16)
    in_maps = []
    for core in range(NCORES):
        b, hh = core // 2, core % 2
        rows = slice(hh * 512, (hh + 1) * 512)
        xt = hidden_state[b].T.astype(np.float16).reshape(DC, P, S)
        # w[rows].T: [1024 d, 512 c] -> (pair, k) chunks [4, 8, 128, 128]
        wqt = (w_q[rows].T.reshape(DC, P, PAIRS, P).transpose(2, 0, 1, 3)
               .astype(np.float16))
        wkt = (w_k[rows].T.reshape(DC, P, PAIRS, P).transpose(2, 0, 1, 3)
               .astype(np.float16))
        wvt = w_v[rows].T.reshape(DC, P, 512).astype(np.float16)
        woth = np.ascontiguousarray(w_o[:, rows].T.reshape(PAIRS, P, D)
                                    ).astype(np.float16)
        in_maps.append({"xt": np.ascontiguousarray(xt),
                        "wqt": np.ascontiguousarray(wqt),
                        "wkt": np.ascontiguousarray(wkt),
                        "wvt": np.ascontiguousarray(wvt),
                        "woth": woth,
                        "ident": ident})
    return in_maps


def _assemble(results):
    out = np.empty((B, S, D), np.float32)
    for b in range(B):
        t = (results[2 * b]["outt"].reshape(D, S).astype(np.float32)
             + results[2 * b + 1]["outt"].reshape(D, S).astype(np.float32))
        out[b] = t.T
    return out


def run_spmd(hidden_state, w_q, w_k, w_v, w_o, **spmd_kwargs):
    """Run the kernel; returns (output, BassKernelResults)."""
    from concourse.bass_utils import run_bass_kernel_spmd

    nc = _get_nc()
    in_maps = _make_in_maps(hidden_state, w_q, w_k, w_v, w_o)
    res = run_bass_kernel_spmd(nc, in_maps, core_ids=list(range(NCORES)),
                               **spmd_kwargs)
    return _assemble(res.results), res


def kernel(hidden_state, attention_mask=None, w_q=None, w_k=None, w_v=None,
           w_o=None):
    out, _ = run_spmd(hidden_state, w_q, w_k, w_v, w_o)
    return out


# revision 11
# speedup vs baseline: 1.1155x; 1.0251x over previous
"""BertAttention Trainium2 kernel (8 NeuronCores, SPMD).

Sharding: core c handles batch b = c//2 and head-half hh = c%2 (8 of 16 heads).
Each core computes q/k/v projections for its 512 head-dims over its batch's
full sequence, per-head attention (no mask, scale 1/sqrt(1024)), and a partial
o-projection over its 512 context dims.  Host sums the two partials per batch.

v2 design (ACT-exp is the hard floor at ~285us/core; PE work reduced below it):
  QK  : unchanged (K=64 row-packed pairs, fp16, half-rate -- irreducible).
  exp : ACT over st [128,2,512] psum -> pt [128,2,512] f16, 256 instrs.
  AV  : SWAPPED dataflow -- stationary = P^T chunk [128k,128q] (from pt),
        moving = V_aug [128k,65] (64 v-dims + ones col). out = ctx [128q,65]
        psum, accumulated over 16 k-tiles per (head j, q-chunk qc).  Full
        128-wide array => ~2x fewer PE cycles than the ctx^T form.  The
        softmax denominator lands in psum COLUMN 64 -> per-partition DVE
        reciprocal + tensor_scalar_mul (no broadcast matmuls, no [64,2048]
        DVE multiplies).
  ctxT: PE transpose (identity matmul) of normalized ctx [128q,64] ->
        [64,128] written at column-position 64j, gpsimd copies psum->sbuf.
  oproj: as baseline (per-pair psum chunks, DVE add into out_sb), fp16 out.

Emission order keeps ACT continuously fed: per (pair p, qt) block, the 16
kt-slots emit QK+exp, and between them: AV chains of the PREVIOUS block
(1 chain per 2 slots), q/k projection bursts for pair p+1, v-projection
half-passes, and o-proj chunks for pair p-1.

PSUM (16KB/partition): st 2x[128,2,512]f32 (8KB) + acc 2x[128,65]f32 padded
to 2KB (4KB) + pp 2x2KB (proj/oproj/transpose, 4KB).
"""

import sys

sys.path.insert(0, "/opt/trn_rl_repo")

import numpy as np

B, S, D, H = 4, 2048, 1024, 16
HEAD = 64
NCORES = 8
P = 128
NQ = 512            # q free-tile width
KT_TILES = S // P   # 16 k tiles
QT_TILES = S // NQ  # 4 q tiles
DC = 8              # contraction chunks for projections (1024/128)
PAIRS = 4           # head pairs per core


_NC_CACHE = None


def _build_nc():
    import concourse.bass as bass  # noqa: F401
    import concourse.tile as tile
    from concourse import bacc, mybir

    f32 = mybir.dt.float32
    f16 = mybir.dt.float16
    AF = mybir.ActivationFunctionType

    nc = bacc.Bacc(None)
    xt_d = nc.declare_dram_parameter("xt", [DC, P, S], f16, isOutput=False)
    wqt_d = nc.declare_dram_parameter("wqt", [PAIRS, DC, P, P], f16, isOutput=False)
    wkt_d = nc.declare_dram_parameter("wkt", [PAIRS, DC, P, P], f16, isOutput=False)
    wvt_d = nc.declare_dram_parameter("wvt", [DC, P, 512], f16, isOutput=False)
    woth_d = nc.declare_dram_parameter("woth", [PAIRS, P, D], f16, isOutput=False)
    ident_d = nc.declare_dram_parameter("ident", [P, P], f16, isOutput=False)
    out_d = nc.declare_dram_parameter("outt", [D // P, P, S], f16, isOutput=True)

    from contextlib import ExitStack

    with tile.TileContext(nc) as tc, ExitStack() as es:
        def pool(name, bufs, space="SBUF"):
            return es.enter_context(
                tc.tile_pool(name=name, bufs=bufs, space=space))

        xt_pool = pool("xt", 1)
        # 16 slots = 2 pairs' weights resident: pair p+1's weight DMAs must
        # not wait on pair p's LAST (late-dribbled) proj burst, since pair
        # p+1's own bursts sit earlier in the in-order PE stream.
        wq_pool = pool("wq", 16)
        wk_pool = pool("wk", 16)
        wv_pool = pool("wv", 8)
        qt_pool = pool("qt", 2)
        kt_pool = pool("kt", 2)
        v_pool = pool("v", 1)
        pt_pool = pool("pt", 32)
        cn_pool = pool("cn", 4)
        rc_pool = pool("rc", 4)
        ctx_pool = pool("ctx", 1)
        wot_pool = pool("wot", 1)
        ost_pool = pool("ost", 1)
        on_pool = pool("on", 1)
        pp_pool = pool("pp", 2, "PSUM")
        st_pool = pool("st", 2, "PSUM")
        acc_pool = pool("acc", 2, "PSUM")

        # identity for PE transposes
        ident = on_pool.tile([P, P], f16, tag="id", name="ident")
        nc.sync.dma_start(ident[:], ident_d[:, :])

        # PE warmup during the initial DMA: keeps HAM at 8/8 so the
        # first projection matmuls run at 2.4 GHz
        wup = on_pool.tile([P, NQ], f16, tag="wup", name="wup")
        nc.vector.memset(wup[:], 0.125)
        wups = pp_pool.tile([P, NQ], f32, tag="pp", name="wups")
        for _ in range(30):
            nc.tensor.matmul(wups[:], wup[:, 0:P], wup[:],
                             start=True, stop=True)

        # load x^T chunks -- spread across 4 engine queues so the first
        # projections (which contract over ALL chunks) start ~4x sooner
        xt = []
        dma_engs = [nc.sync, nc.gpsimd, nc.scalar]
        for k in range(DC):
            t = xt_pool.tile([P, S], f16, tag=f"xt{k}", name=f"xt{k}")
            dma_engs[k % 3].dma_start(t[:], xt_d[k])
            xt.append(t)

        wv_t = []
        for k in range(DC):
            t = wv_pool.tile([P, NQ], f16, tag="wv", name="wv")
            dma_engs[(k + 1) % 3].dma_start(t[:], wvt_d[k])
            wv_t.append(t)

        # V_aug tiles per k-tile: [128 keys, 4 heads, 65] (64 v-dims + ones)
        v_half = {0: [None] * KT_TILES, 1: [None] * KT_TILES}

        def proj_v(mt, half):
            """one N=256 projection pass filling v_half[half][mt]"""
            ps = pp_pool.tile([P, 256], f32, tag="pp", name="ppv")
            for k in range(DC):
                nc.tensor.matmul(
                    ps[:], xt[k][:, mt * P:(mt + 1) * P],
                    wv_t[k][:, half * 256:(half + 1) * 256],
                    start=(k == 0), stop=(k == DC - 1),
                )
            t = v_pool.tile([P, 4, 65], f16, tag=f"v{half}_{mt}",
                            name=f"v{half}_{mt}")
            nc.vector.memset(t[:], 1.0)
            v_half[half][mt] = t
            src = ps[:, :].rearrange("p (h d) -> p h d", h=4)
            nc.vector.tensor_copy(t[:, :, 0:64], src)

        def load_w(w_pool, w_dram, p):
            w_t = []
            for k in range(DC):
                t = w_pool.tile([P, P], f16, tag="w", name="w")
                nc.sync.dma_start(t[:], w_dram[p, k])
                w_t.append(t)
            return w_t

        def proj_nt(out, w_t, nt):
            ps = pp_pool.tile([P, NQ], f32, tag="pp", name="pp")
            for k in range(DC):
                nc.tensor.matmul(
                    ps[:], w_t[k][:], xt[k][:, nt * NQ:(nt + 1) * NQ],
                    start=(k == 0), stop=(k == DC - 1),
                )
            nc.vector.tensor_copy(out[:, nt * NQ:(nt + 1) * NQ], ps[:])

        # ---- per-pair state ----
        QTs, KTs, ctxTs = {}, {}, {}
        wot_t = {}
        wqk_t = {}
        out_sb = []
        for _ot in range(D // P):
            _t = ost_pool.tile([P, S], f16, tag=f"ou{_ot}", name=f"ou{_ot}")
            out_sb.append(_t)

        def load_wot(p):
            th = wot_pool.tile([P, D], f16, tag=f"woth{p}", name=f"woth{p}")
            nc.gpsimd.dma_start(th[:], woth_d[p])
            wot_t[p] = th

        def alloc_pair(p):
            KTs[p] = kt_pool.tile([P, S], f16, tag="t", name=f"kt{p}")
            QTs[p] = qt_pool.tile([P, S], f16, tag="t", name=f"qt{p}")
            ctxTs[p] = ctx_pool.tile([P, S], f16, tag=f"ctx{p}",
                                     name=f"ctx{p}")

        def oproj_chunk(p, qt):
            for ot in range(D // P):
                ps = pp_pool.tile([P, NQ], f32, tag="pp", name="pp")
                nc.tensor.matmul(
                    ps[:], wot_t[p][:, ot * P:(ot + 1) * P],
                    ctxTs[p][:, qt * NQ:(qt + 1) * NQ],
                    start=True, stop=True,
                )
                dst = out_sb[ot][:, qt * NQ:(qt + 1) * NQ]
                if p == 0:
                    nc.vector.tensor_copy(dst, ps[:])
                else:
                    nc.vector.tensor_add(dst, dst, ps[:])
                if p == PAIRS - 1:
                    eng = nc.sync if ot % 2 == 0 else nc.gpsimd
                    eng.dma_start(out_d[ot][:, qt * NQ:(qt + 1) * NQ], dst)

        # pt tiles of the two most recent blocks
        pt_map = {}

        tp_box = [None]
        # deferred transpose: (bp, bqt, c, ctxn) emitted one chain later so
        # the PE never stalls on the DVE normalize (measured ~1.1us/chain)
        pend_tr = [None]

        def flush_transpose():
            if pend_tr[0] is None:
                return
            bp, bqt, c, ctxn = pend_tr[0]
            pend_tr[0] = None
            j, qc = c % 2, c // 2
            if j == 0:
                tp_box[0] = pp_pool.tile([P, P], f16, tag="pp", name="tp")
            tp = tp_box[0]
            nc.tensor.matmul(tp[64 * j:64 * (j + 1), :], ctxn[:], ident[:],
                             is_transpose=True)
            if j == 1:
                # gpsimd cannot read PSUM; DVE does the psum->sbuf hop
                nc.vector.tensor_copy(
                    ctxTs[bp][:, bqt * NQ + qc * P: bqt * NQ + (qc + 1) * P],
                    tp[:],
                )

        def av_chain(bp, bqt, c):
            """AV chain c (j = c%2, qc = c//2) of block (bp, bqt):
            ctx[q,d] accumulation + normalize; transpose deferred."""
            j, qc = c % 2, c // 2
            half = bp // 2
            jj = (2 * bp + j) % 4
            acc = acc_pool.tile([P, 65], f32, tag="acc", name="acc",
                                padded_shape=[P, 512])
            for i in range(KT_TILES):
                nc.tensor.matmul(
                    acc[:],
                    pt_map[(bp, bqt, i)][:, j, qc * P:(qc + 1) * P],
                    v_half[half][i][:, jj, :],
                    start=(i == 0), stop=(i == KT_TILES - 1),
                    skip_group_check=True,
                )
            flush_transpose()
            rec = rc_pool.tile([P, 1], f32, tag="rc", name="rec")
            nc.vector.reciprocal_approx_fast(rec[:], acc[:, 64:65])
            ctxn = cn_pool.tile([P, 64], f16, tag="cn", name="ctxn")
            nc.vector.tensor_scalar_mul(ctxn[:], acc[:, 0:64], rec[:, 0:1])
            pend_tr[0] = (bp, bqt, c, ctxn)

        def qk_slot(p, qt, kt):
            st = st_pool.tile([P, 2, NQ], f32, tag="st", name="st")
            for j in range(2):
                h = j * 64
                nc.tensor.matmul(
                    st[:, j, :],
                    KTs[p][h:h + 64, kt * P:(kt + 1) * P],
                    QTs[p][h:h + 64, qt * NQ:(qt + 1) * NQ],
                    start=True, stop=True,
                )
            pt = pt_pool.tile([P, 2, NQ], f16, tag="pt", name="pt")
            nc.scalar.activation(pt[:], st[:], AF.Exp, scale=0.03125)
            pt_map[(p, qt, kt)] = pt

        # ---------------- schedule ----------------
        # extra-work placement: dict (p, qt, kt) -> list of callables
        extra = {}

        def put(p, qt, kt, fn):
            extra.setdefault((p, qt, kt), []).append(fn)

        # pair 0 projections: K nt0 + Q nt0 upfront; rest dribbled in (0,0)
        alloc_pair(0)
        wqk_t[0] = (load_w(wk_pool, wkt_d, 0), load_w(wq_pool, wqt_d, 0))
        proj_nt(KTs[0], wqk_t[0][0], 0)
        proj_nt(QTs[0], wqk_t[0][1], 0)
        # remaining K-chunks of pair 0 needed at kt = 4*nt of every block
        for ntc in (1, 2, 3):
            put(0, 0, 4 * (ntc - 1) + 1,
                lambda ntc=ntc: proj_nt(KTs[0], wqk_t[0][0], ntc))
        # Q-chunks of pair 0: Qnt_c needed from block (0, c)
        for ntc in (1, 2, 3):
            put(0, ntc - 1, 6, lambda ntc=ntc: proj_nt(QTs[0], wqk_t[0][1], ntc))
        # v-projection half 0 (pairs 0,1): all 16 passes in block (0,0) --
        # the first AV chains (of block (0,0), run in block (0,1)) need
        # every v tile, so none may be emitted later.
        for mt in range(KT_TILES):
            put(0, 0, mt, lambda mt=mt: proj_v(mt, 0))
        # v-projection half 1 (pairs 2,3): spread over pair-1 blocks
        for mt in range(KT_TILES):
            put(1, mt // 4, (mt % 4) * 4 + 2, lambda mt=mt: proj_v(mt, 1))

        # pair p+1 q/k projections: 8 bursts spread over pair-p blocks
        for p in range(PAIRS - 1):
            pn = p + 1

            def loadw(pn=pn):
                alloc_pair(pn)
                wqk_t[pn] = (load_w(wk_pool, wkt_d, pn),
                             load_w(wq_pool, wqt_d, pn))
            put(p, 0, 0, loadw)
            for i in range(4):  # K chunks first
                put(p, 1 + i % 3, 3 + 2 * (i // 3),
                    lambda pn=pn, i=i: proj_nt(KTs[pn], wqk_t[pn][0], i))
            for i in range(4):
                put(p, 1 + i % 3, 9 + 2 * (i // 3),
                    lambda pn=pn, i=i: proj_nt(QTs[pn], wqk_t[pn][1], i))

        # wot loads + o-proj chunks for pair p-1 during pair p's blocks
        put(0, 2, 1, lambda: load_wot(0))
        put(1, 0, 1, lambda: load_wot(1))
        put(2, 0, 1, lambda: load_wot(2))
        put(2, 2, 1, lambda: load_wot(3))
        for p in range(1, PAIRS):
            for qt in range(QT_TILES):
                put(p, qt, 13, lambda p=p, qt=qt: oproj_chunk(p - 1, qt))
        # pair 3's own o-proj: chunk qt right after its chains complete
        # (must flush the pending chain-7 transpose first)
        for qt in range(1, QT_TILES):
            put(3, qt, 15,
                lambda qt=qt: (flush_transpose(), oproj_chunk(3, qt - 1)))

        blocks = [(p, qt) for p in range(PAIRS) for qt in range(QT_TILES)]
        for bi, (p, qt) in enumerate(blocks):
            prev = blocks[bi - 1] if bi > 0 else None
            for kt in range(KT_TILES):
                qk_slot(p, qt, kt)
                if prev is not None and kt % 2 == 1:
                    av_chain(prev[0], prev[1], kt // 2)
                for fn in extra.get((p, qt, kt), ()):
                    fn()
            if prev is not None:
                # release prev block's pt tiles from the map
                for kk in range(KT_TILES):
                    del pt_map[(prev[0], prev[1], kk)]
        # tail: chains of the last block + final o-proj chunk
        for c in range(8):
            av_chain(3, 3, c)
        flush_transpose()
        oproj_chunk(3, 3)

    nc.finalize()
    return nc


def _get_nc():
    global _NC_CACHE
    if _NC_CACHE is None:
        _NC_CACHE = _build_nc()
    return _NC_CACHE


def _make_in_maps(hidden_state, w_q, w_k, w_v, w_o):
    hidden_state = np.asarray(hidden_state, np.float32)
    w_q = np.asarray(w_q, np.float32)
    w_k = np.asarray(w_k, np.float32)
    w_v = np.asarray(w_v, np.float32)
    w_o = np.asarray(w_o, np.float32)

    ident = np.eye(P, dtype=np.float16)
    in_maps = []
    for core in range(NCORES):
        b, hh = core // 2, core % 2
        rows = slice(hh * 512, (hh + 1) * 512)
        xt = hidden_state[b].T.astype(np.float16).reshape(DC, P, S)
        # w[rows].T: [1024 d, 512 c] -> (pair, k) chunks [4, 8, 128, 128]
        wqt = (w_q[rows].T.reshape(DC, P, PAIRS, P).transpose(2, 0, 1, 3)
               .astype(np.float16))
        wkt = (w_k[rows].T.reshape(DC, P, PAIRS, P).transpose(2, 0, 1, 3)
               .astype(np.float16))
        wvt = w_v[rows].T.reshape(DC, P, 512).astype(np.float16)
        woth = np.ascontiguousarray(w_o[:, rows].T.reshape(PAIRS, P, D)
                                    ).astype(np.float16)
        in_maps.append({"xt": np.ascontiguousarray(xt),
                        "wqt": np.ascontiguousarray(wqt),
                        "wkt": np.ascontiguousarray(wkt),
                        "wvt": np.ascontiguousarray(wvt),
                        "woth": woth,
                        "ident": ident})
    return in_maps


def _assemble(results):
    out = np.empty((B, S, D), np.float32)
    for b in range(B):
        t = (results[2 * b]["outt"].reshape(D, S).astype(np.float32)
             + results[2 * b + 1]["outt"].reshape(D, S).astype(np.float32))
        out[b] = t.T
    return out


def run_spmd(hidden_state, w_q, w_k, w_v, w_o, **spmd_kwargs):
    """Run the kernel; returns (output, BassKernelResults)."""
    from concourse.bass_utils import run_bass_kernel_spmd

    nc = _get_nc()
    in_maps = _make_in_maps(hidden_state, w_q, w_k, w_v, w_o)
    res = run_bass_kernel_spmd(nc, in_maps, core_ids=list(range(NCORES)),
                               **spmd_kwargs)
    return _assemble(res.results), res


def kernel(hidden_state, attention_mask=None, w_q=None, w_k=None, w_v=None,
           w_o=None):
    out, _ = run_spmd(hidden_state, w_q, w_k, w_v, w_o)
    return out


# revision 18
# speedup vs baseline: 1.1233x; 1.0070x over previous
"""BertAttention Trainium2 kernel (8 NeuronCores, SPMD).

Sharding: core c handles batch b = c//2 and head-half hh = c%2 (8 of 16 heads).
Each core computes q/k/v projections for its 512 head-dims over its batch's
full sequence, per-head attention (no mask, scale 1/sqrt(1024)), and a partial
o-projection over its 512 context dims.  Host sums the two partials per batch.

v2 design (ACT-exp is the hard floor at ~285us/core; PE work reduced below it):
  QK  : unchanged (K=64 row-packed pairs, fp16, half-rate -- irreducible).
  exp : ACT over st [128,2,512] psum -> pt [128,2,512] f16, 256 instrs.
  AV  : SWAPPED dataflow -- stationary = P^T chunk [128k,128q] (from pt),
        moving = V_aug [128k,65] (64 v-dims + ones col). out = ctx [128q,65]
        psum, accumulated over 16 k-tiles per (head j, q-chunk qc).  Full
        128-wide array => ~2x fewer PE cycles than the ctx^T form.  The
        softmax denominator lands in psum COLUMN 64 -> per-partition DVE
        reciprocal + tensor_scalar_mul (no broadcast matmuls, no [64,2048]
        DVE multiplies).
  ctxT: PE transpose (identity matmul) of normalized ctx [128q,64] ->
        [64,128] written at column-position 64j, gpsimd copies psum->sbuf.
  oproj: as baseline (per-pair psum chunks, DVE add into out_sb), fp16 out.

Emission order keeps ACT continuously fed: per (pair p, qt) block, the 16
kt-slots emit QK+exp, and between them: AV chains of the PREVIOUS block
(1 chain per 2 slots), q/k projection bursts for pair p+1, v-projection
half-passes, and o-proj chunks for pair p-1.

PSUM (16KB/partition): st 2x[128,2,512]f32 (8KB) + acc 2x[128,65]f32 padded
to 2KB (4KB) + pp 2x2KB (proj/oproj/transpose, 4KB).
"""

import sys

sys.path.insert(0, "/opt/trn_rl_repo")

import numpy as np

B, S, D, H = 4, 2048, 1024, 16
HEAD = 64
NCORES = 8
P = 128
NQ = 512            # q free-tile width
KT_TILES = S // P   # 16 k tiles
QT_TILES = S // NQ  # 4 q tiles
DC = 8              # contraction chunks for projections (1024/128)
PAIRS = 4           # head pairs per core


_NC_CACHE = None


def _build_nc():
    import concourse.bass as bass  # noqa: F401
    import concourse.tile as tile
    from concourse import bacc, mybir

    f32 = mybir.dt.float32
    f16 = mybir.dt.float16
    AF = mybir.ActivationFunctionType

    nc = bacc.Bacc(None)
    xt_d = nc.declare_dram_parameter("xt", [DC, P, S], f16, isOutput=False)
    # per-pair contiguous weight images [P, DC*P]: one big DMA each (the
    # [DC,P,P] chunk layout produced 256B/partition packets, ~45us to land)
    wqt_d = nc.declare_dram_parameter("wqt", [PAIRS, P, DC * P], f16, isOutput=False)
    wkt_d = nc.declare_dram_parameter("wkt", [PAIRS, P, DC * P], f16, isOutput=False)
    wvt_d = nc.declare_dram_parameter("wvt", [DC, P, 512], f16, isOutput=False)
    woth_d = nc.declare_dram_parameter("woth", [PAIRS, P, D], f16, isOutput=False)
    ident_d = nc.declare_dram_parameter("ident", [P, P], f16, isOutput=False)
    out_d = nc.declare_dram_parameter("outt", [D // P, P, S], f16, isOutput=True)

    from contextlib import ExitStack

    with tile.TileContext(nc) as tc, ExitStack() as es:
        def pool(name, bufs, space="SBUF"):
            return es.enter_context(
                tc.tile_pool(name=name, bufs=bufs, space=space))

        xt_pool = pool("xt", 1)
        # 4 slots = all pairs' weights resident: pair p+1's weight DMA must
        # not wait on pair p's LAST (late-dribbled) proj burst, since pair
        # p+1's own bursts sit earlier in the in-order PE stream.
        wq_pool = pool("wq", 2)
        wk_pool = pool("wk", 2)
        wv_pool = pool("wv", 8)
        qt_pool = pool("qt", 2)
        kt_pool = pool("kt", 2)
        v_pool = pool("v", 1)
        pt_pool = pool("pt", 32)
        cn_pool = pool("cn", 4)
        rc_pool = pool("rc", 4)
        ctx_pool = pool("ctx", 1)
        wot_pool = pool("wot", 1)
        ost_pool = pool("ost", 1)
        on_pool = pool("on", 1)
        pp_pool = pool("pp", 2, "PSUM")
        st_pool = pool("st", 2, "PSUM")
        acc_pool = pool("acc", 2, "PSUM")

        # PE warmup during the initial DMA: keeps HAM at 8/8 so the
        # first projection matmuls run at 2.4 GHz
        wup = on_pool.tile([P, NQ], f16, tag="wup", name="wup")
        nc.vector.memset(wup[:], 0.125)
        wups = pp_pool.tile([P, NQ], f32, tag="pp", name="wups")
        for _ in range(30):
            nc.tensor.matmul(wups[:], wup[:, 0:P], wup[:],
                             start=True, stop=True)

        # load x^T chunks -- spread across 4 engine queues so the first
        # projections (which contract over ALL chunks) start ~4x sooner
        xt = []
        dma_engs = [nc.sync, nc.gpsimd, nc.scalar]
        for k in range(DC):
            t = xt_pool.tile([P, S], f16, tag=f"xt{k}", name=f"xt{k}")
            dma_engs[k % 3].dma_start(t[:], xt_d[k])
            xt.append(t)

        wv_t = []
        for k in range(DC):
            t = wv_pool.tile([P, NQ], f16, tag="wv", name="wv")
            dma_engs[(k + 1) % 3].dma_start(t[:], wvt_d[k])
            wv_t.append(t)

        # identity for PE transposes (after xt/wv in the queue)
        ident = on_pool.tile([P, P], f16, tag="id", name="ident")
        nc.gpsimd.dma_start(ident[:], ident_d[:, :])

        # V_aug tiles per k-tile: [128 keys, 4 heads, 65] (64 v-dims + ones)
        v_half = {0: [None] * KT_TILES, 1: [None] * KT_TILES}

        def proj_v(mt, half):
            """one N=256 projection pass filling v_half[half][mt]"""
            ps = pp_pool.tile([P, 256], f32, tag="pp", name="ppv")
            for k in range(DC):
                nc.tensor.matmul(
                    ps[:], xt[k][:, mt * P:(mt + 1) * P],
                    wv_t[k][:, half * 256:(half + 1) * 256],
                    start=(k == 0), stop=(k == DC - 1),
                )
            t = v_pool.tile([P, 4, 65], f16, tag=f"v{half}_{mt}",
                            name=f"v{half}_{mt}")
            nc.vector.memset(t[:], 1.0)
            v_half[half][mt] = t
            src = ps[:, :].rearrange("p (h d) -> p h d", h=4)
            nc.vector.tensor_copy(t[:, :, 0:64], src)

        def load_w(w_pool, w_dram, p):
            t = w_pool.tile([P, DC * P], f16, tag="w", name="w")
            nc.sync.dma_start(t[:], w_dram[p])
            return [t[:, k * P:(k + 1) * P] for k in range(DC)]

        def proj_nt(out, w_t, nt):
            ps = pp_pool.tile([P, NQ], f32, tag="pp", name="pp")
            for k in range(DC):
                nc.tensor.matmul(
                    ps[:], w_t[k][:], xt[k][:, nt * NQ:(nt + 1) * NQ],
                    start=(k == 0), stop=(k == DC - 1),
                )
            nc.vector.tensor_copy(out[:, nt * NQ:(nt + 1) * NQ], ps[:])

        # ---- per-pair state ----
        QTs, KTs, ctxTs = {}, {}, {}
        wot_t = {}
        wqk_t = {}
        out_sb = []
        for _ot in range(D // P):
            _t = ost_pool.tile([P, S], f16, tag=f"ou{_ot}", name=f"ou{_ot}")
            out_sb.append(_t)

        def load_wot(p):
            th = wot_pool.tile([P, D], f16, tag=f"woth{p}", name=f"woth{p}")
            nc.gpsimd.dma_start(th[:], woth_d[p])
            wot_t[p] = th

        def alloc_pair(p):
            KTs[p] = kt_pool.tile([P, S], f16, tag="t", name=f"kt{p}")
            QTs[p] = qt_pool.tile([P, S], f16, tag="t", name=f"qt{p}")
            ctxTs[p] = ctx_pool.tile([P, S], f16, tag=f"ctx{p}",
                                     name=f"ctx{p}")

        def oproj_chunk(p, qt):
            for ot in range(D // P):
                ps = pp_pool.tile([P, NQ], f32, tag="pp", name="pp")
                nc.tensor.matmul(
                    ps[:], wot_t[p][:, ot * P:(ot + 1) * P],
                    ctxTs[p][:, qt * NQ:(qt + 1) * NQ],
                    start=True, stop=True,
                )
                dst = out_sb[ot][:, qt * NQ:(qt + 1) * NQ]
                if p == 0:
                    nc.vector.tensor_copy(dst, ps[:])
                else:
                    nc.vector.tensor_add(dst, dst, ps[:])
                if p == PAIRS - 1:
                    eng = nc.sync if ot % 2 == 0 else nc.gpsimd
                    eng.dma_start(out_d[ot][:, qt * NQ:(qt + 1) * NQ], dst)

        # pt tiles of the two most recent blocks
        pt_map = {}

        tp_box = [None]
        # deferred transpose: (bp, bqt, c, ctxn) emitted one chain later so
        # the PE never stalls on the DVE normalize (measured ~1.1us/chain)
        pend_tr = [None]

        def flush_transpose():
            if pend_tr[0] is None:
                return
            bp, bqt, c, ctxn = pend_tr[0]
            pend_tr[0] = None
            j, qc = c % 2, c // 2
            if j == 0:
                tp_box[0] = pp_pool.tile([P, P], f16, tag="pp", name="tp")
            tp = tp_box[0]
            nc.tensor.matmul(tp[64 * j:64 * (j + 1), :], ctxn[:], ident[:],
                             is_transpose=True)
            if j == 1:
                # gpsimd cannot read PSUM; DVE does the psum->sbuf hop
                nc.vector.tensor_copy(
                    ctxTs[bp][:, bqt * NQ + qc * P: bqt * NQ + (qc + 1) * P],
                    tp[:],
                )

        def av_chain(bp, bqt, c):
            """AV chain c (j = c%2, qc = c//2) of block (bp, bqt):
            ctx[q,d] accumulation + normalize; transpose deferred."""
            j, qc = c % 2, c // 2
            half = bp // 2
            jj = (2 * bp + j) % 4
            acc = acc_pool.tile([P, 65], f32, tag="acc", name="acc",
                                padded_shape=[P, 512])
            for i in range(KT_TILES):
                nc.tensor.matmul(
                    acc[:],
                    pt_map[(bp, bqt, i)][:, j, qc * P:(qc + 1) * P],
                    v_half[half][i][:, jj, :],
                    start=(i == 0), stop=(i == KT_TILES - 1),
                    skip_group_check=True,
                )
            flush_transpose()
            rec = rc_pool.tile([P, 1], f32, tag="rc", name="rec")
            nc.vector.reciprocal_approx_fast(rec[:], acc[:, 64:65])
            ctxn = cn_pool.tile([P, 64], f16, tag="cn", name="ctxn")
            nc.vector.tensor_scalar_mul(ctxn[:], acc[:, 0:64], rec[:, 0:1])
            pend_tr[0] = (bp, bqt, c, ctxn)

        def qk_slot(p, qt, kt):
            st = st_pool.tile([P, 2, NQ], f32, tag="st", name="st")
            for j in range(2):
                h = j * 64
                nc.tensor.matmul(
                    st[:, j, :],
                    KTs[p][h:h + 64, kt * P:(kt + 1) * P],
                    QTs[p][h:h + 64, qt * NQ:(qt + 1) * NQ],
                    start=True, stop=True,
                )
            pt = pt_pool.tile([P, 2, NQ], f16, tag="pt", name="pt")
            nc.scalar.activation(pt[:], st[:], AF.Exp, scale=0.03125)
            pt_map[(p, qt, kt)] = pt

        # ---------------- schedule ----------------
        # extra-work placement: dict (p, qt, kt) -> list of callables
        extra = {}

        def put(p, qt, kt, fn):
            extra.setdefault((p, qt, kt), []).append(fn)

        # pair 0 projections: K nt0 + Q nt0 upfront; rest dribbled in (0,0)
        alloc_pair(0)
        wqk_t[0] = (load_w(wk_pool, wkt_d, 0), load_w(wq_pool, wqt_d, 0))
        proj_nt(KTs[0], wqk_t[0][0], 0)
        proj_nt(QTs[0], wqk_t[0][1], 0)
        # remaining K-chunks of pair 0 needed at kt = 4*nt of every block
        for ntc in (1, 2, 3):
            put(0, 0, 4 * (ntc - 1) + 1,
                lambda ntc=ntc: proj_nt(KTs[0], wqk_t[0][0], ntc))
        # Q-chunks of pair 0: Qnt_c needed from block (0, c)
        for ntc in (1, 2, 3):
            put(0, ntc - 1, 6, lambda ntc=ntc: proj_nt(QTs[0], wqk_t[0][1], ntc))
        # v-projection half 0 (pairs 0,1): all 16 passes in block (0,0) --
        # the first AV chains (of block (0,0), run in block (0,1)) need
        # every v tile, so none may be emitted later.
        for mt in range(KT_TILES):
            put(0, 0, mt, lambda mt=mt: proj_v(mt, 0))
        # v-projection half 1 (pairs 2,3): spread over pair-1 blocks
        for mt in range(KT_TILES):
            put(1, mt // 4, (mt % 4) * 4 + 2, lambda mt=mt: proj_v(mt, 1))

        # pair p+1 q/k projections: 8 bursts spread over pair-p blocks
        for p in range(PAIRS - 1):
            pn = p + 1

            def loadw(pn=pn):
                alloc_pair(pn)
                wqk_t[pn] = (load_w(wk_pool, wkt_d, pn),
                             load_w(wq_pool, wqt_d, pn))
            put(p, 0, 0, loadw)
            for i in range(4):  # K chunks first
                put(p, 1 + i % 3, 3 + 2 * (i // 3),
                    lambda pn=pn, i=i: proj_nt(KTs[pn], wqk_t[pn][0], i))
            for i in range(4):
                put(p, 1 + i % 3, 9 + 2 * (i // 3),
                    lambda pn=pn, i=i: proj_nt(QTs[pn], wqk_t[pn][1], i))

        # wot loads + o-proj chunks for pair p-1 during pair p's blocks
        put(0, 2, 1, lambda: load_wot(0))
        put(1, 0, 1, lambda: load_wot(1))
        put(2, 0, 1, lambda: load_wot(2))
        put(2, 2, 1, lambda: load_wot(3))
        for p in range(1, PAIRS):
            for qt in range(QT_TILES):
                put(p, qt, 13, lambda p=p, qt=qt: oproj_chunk(p - 1, qt))
        # pair 3's own o-proj: chunk qt right after its chains complete
        # (must flush the pending chain-7 transpose first)
        for qt in range(1, QT_TILES):
            put(3, qt, 15,
                lambda qt=qt: (flush_transpose(), oproj_chunk(3, qt - 1)))

        blocks = [(p, qt) for p in range(PAIRS) for qt in range(QT_TILES)]
        for bi, (p, qt) in enumerate(blocks):
            prev = blocks[bi - 1] if bi > 0 else None
            for kt in range(KT_TILES):
                qk_slot(p, qt, kt)
                if prev is not None and kt % 2 == 1:
                    av_chain(prev[0], prev[1], kt // 2)
                for fn in extra.get((p, qt, kt), ()):
                    fn()
            if prev is not None:
                # release prev block's pt tiles from the map
                for kk in range(KT_TILES):
                    del pt_map[(prev[0], prev[1], kk)]
        # tail: chains of the last block + final o-proj chunk
        for c in range(8):
            av_chain(3, 3, c)
        flush_transpose()
        oproj_chunk(3, 3)

    nc.finalize()
    return nc


def _get_nc():
    global _NC_CACHE
    if _NC_CACHE is None:
        _NC_CACHE = _build_nc()
    return _NC_CACHE


def _make_in_maps(hidden_state, w_q, w_k, w_v, w_o):
    hidden_state = np.asarray(hidden_state, np.float32)
    w_q = np.asarray(w_q, np.float32)
    w_k = np.asarray(w_k, np.float32)
    w_v = np.asarray(w_v, np.float32)
    w_o = np.asarray(w_o, np.float32)

    ident = np.eye(P, dtype=np.float16)
    in_maps = []
    for core in range(NCORES):
        b, hh = core // 2, core % 2
        rows = slice(hh * 512, (hh + 1) * 512)
        xt = hidden_state[b].T.astype(np.float16).reshape(DC, P, S)
        # w[rows].T: [1024 d, 512 c] -> per-pair contiguous [4, 128, 8*128]
        wqt = (w_q[rows].T.reshape(DC, P, PAIRS, P).transpose(2, 1, 0, 3)
               .reshape(PAIRS, P, DC * P).astype(np.float16))
        wkt = (w_k[rows].T.reshape(DC, P, PAIRS, P).transpose(2, 1, 0, 3)
               .reshape(PAIRS, P, DC * P).astype(np.float16))
        wvt = w_v[rows].T.reshape(DC, P, 512).astype(np.float16)
        woth = np.ascontiguousarray(w_o[:, rows].T.reshape(PAIRS, P, D)
                                    ).astype(np.float16)
        in_maps.append({"xt": np.ascontiguousarray(xt),
                        "wqt": np.ascontiguousarray(wqt),
                        "wkt": np.ascontiguousarray(wkt),
                        "wvt": np.ascontiguousarray(wvt),
                        "woth": woth,
                        "ident": ident})
    return in_maps


def _assemble(results):
    out = np.empty((B, S, D), np.float32)
    for b in range(B):
        t = (results[2 * b]["outt"].reshape(D, S).astype(np.float32)
             + results[2 * b + 1]["outt"].reshape(D, S).astype(np.float32))
        out[b] = t.T
    return out


def run_spmd(hidden_state, w_q, w_k, w_v, w_o, **spmd_kwargs):
    """Run the kernel; returns (output, BassKernelResults)."""
    from concourse.bass_utils import run_bass_kernel_spmd

    nc = _get_nc()
    in_maps = _make_in_maps(hidden_state, w_q, w_k, w_v, w_o)
    res = run_bass_kernel_spmd(nc, in_maps, core_ids=list(range(NCORES)),
                               **spmd_kwargs)
    return _assemble(res.results), res


def kernel(hidden_state, attention_mask=None, w_q=None, w_k=None, w_v=None,
           w_o=None):
    out, _ = run_spmd(hidden_state, w_q, w_k, w_v, w_o)
    return out
